# revision 6
# baseline (speedup 1.0000x reference)
"""Trainium2 Bass kernel for nn_LogicConvSparseMatrix.

Math: the reference's 15-term weighted logic-op sum collapses to

    out[b,k] = C_ab[k]*A*B + C_a[k]*A + C_b[k]*B + C_1[k]

where A = x[b, ca_k, ha_k+oh, wa_k+ow], B = x[b, cb_k, hb_k+oh, wb_k+ow]
are shifted 126x126 windows.  With alpha = C_b/C_ab, gamma = C_1 -
C_a*C_b/C_ab this factors into

    out = (A + alpha) * (C_ab*B + C_a) + gamma

i.e. one ScalarE affine, one VectorE scalar_tensor_tensor, one GPSIMD
scalar add per kernel k.  The index pairs are known at kernel-build time,
so every gather is a compile-time SBUF view: x is staged as
X[p=h, (c,b,w)] and windows are (partition-range, free-offset) views.

Hardware constraint: compute-engine SBUF operands may only start at
partition 0/32/64/96.  The relative h-shift between the A and B windows
is realized with per-channel shifted column copies made by SBUF->SBUF
DMA (DMA may address any partition), always shifting the smaller-h side
down (S[p] = x[p-s]) so all compute APs start at partition 0.  Output
rows are selected by the store DMA (partition offset h_base).

Sharding: data-parallel over batch, 2 batch items per core, 8 cores.
"""

import numpy as np

B, C, H, W = 16, 64, 128, 128
K = 128
RH = RW = 3
OH, OW = H - RH + 1, W - RW + 1
NCORES = 8
BPC = B // NCORES


def _coeffs(weights):
    """Per-kernel coefficients of out = Cab*a*b + Ca*a + Cb*b + C1."""
    w = [weights[:, i].astype(np.float64) for i in range(16)]
    cab = w[1] - w[2] - w[4] - 2 * w[6] - w[7] + w[8] + 2 * w[9] + w[11] + w[13] - w[14]
    ca = w[2] + w[3] + w[6] + w[7] - w[8] - w[9] - w[12] - w[13]
    cb = w[4] + w[5] + w[6] + w[7] - w[8] - w[9] - w[10] - w[11]
    c1 = w[8] + w[9] + w[10] + w[11] + w[12] + w[13] + w[14] + w[15]
    return cab, ca, cb, c1


def _plan(pairs_a, pairs_b, weights):
    """Host-side per-k schedule: operand sources, shifts, op path, scalars."""
    cab, ca, cb, c1 = _coeffs(weights)
    cols = {}  # (s, c) -> column index in the shifted-copy tile
    plans = []
    for k in range(K):
        ha, wa, cca = int(pairs_a[k][0]), int(pairs_a[k][1]), int(pairs_a[k][2])
        hb, wb, ccb = int(pairs_b[k][0]), int(pairs_b[k][1]), int(pairs_b[k][2])
        base = max(ha, hb)
        # side descriptors: (from_shifted, col_or_chan, w_off)
        if ha >= hb:
            s = ha - hb
            a_src = (False, cca, wa)
            b_src = (False, ccb, wb) if s == 0 else (True, cols.setdefault((s, ccb), len(cols)), wb)
        else:
            s = hb - ha
            b_src = (False, ccb, wb)
            a_src = (True, cols.setdefault((s, cca), len(cols)), wa)

        kab, kka, kkb, kk1 = float(cab[k]), float(ca[k]), float(cb[k]), float(c1[k])
        if abs(kab) <= 1e-7:
            path = "linear"  # drop the (tiny) ab term: |err| <= 1e-7
            scal = (kka, kkb, kk1)
        elif abs(kkb) <= 50.0 * abs(kab) and abs(kka * kkb) <= 50.0 * abs(kab):
            path = "fact"
            scal = (kab, kka, kkb / kab, kk1 - kka * kkb / kab)
        else:
            path = "exact"  # ill-conditioned factorization: 4-op exact path
            scal = (kab, kka, kkb, kk1)
        plans.append((k, base, a_src, b_src, path, scal))
    return plans, cols


def _build(pairs_a, pairs_b, weights):
    import concourse.bacc as bacc
    import concourse.bass as bass  # noqa: F401
    import concourse.mybir as mybir
    from concourse.tile import TileContext

    f32 = mybir.dt.float32
    Copy = mybir.ActivationFunctionType.Copy
    add, mult = mybir.AluOpType.add, mybir.AluOpType.mult

    plans, cols = _plan(pairs_a, pairs_b, weights)
    ncols = max(1, len(cols))

    # SBUF budget (bytes/partition, ~207.9 KiB usable): X 64K + ncols K + slots
    if ncols <= 99:
        bufs_t, bufs_b2 = 6, 4
    elif ncols <= 120:
        bufs_t, bufs_b2 = 3, 2
    else:
        raise RuntimeError(f"shifted-column budget exceeded: {ncols}")

    nc = bacc.Bacc()
    x = nc.dram_tensor("x", [BPC, C, H, W], f32, kind="ExternalInput")
    out = nc.dram_tensor("out", [BPC, K, OH, OW], f32, kind="ExternalOutput")

    with TileContext(nc) as tc:
        with (
            tc.tile_pool(name="xp", bufs=1) as xp,
            tc.tile_pool(name="sp", bufs=1) as sp,
            tc.tile_pool(name="bp", bufs=bufs_b2) as bp,
            tc.tile_pool(name="tp", bufs=bufs_t) as tp,
        ):
            X = xp.tile([H, C * BPC * W], f32)
            Xv = X.rearrange("p (c b w) -> p c b w", c=C, b=BPC)
            # stage x as [h, (c, b, w)]; one DMA per batch item (3-dim APs)
            for b in range(BPC):
                nc.sync.dma_start(out=Xv[:, :, b], in_=x[b].rearrange("c h w -> h c w"))

            S = sp.tile([H, ncols * BPC * W], f32)
            Sv = S.rearrange("p (j b w) -> p j b w", j=ncols, b=BPC)
            for (s, c), j in cols.items():
                # S[p] = x[c, p - s]; head rows [0,s) filled with finite junk
                nc.sync.dma_start(out=Sv[s:H, j], in_=Xv[0 : H - s, c])
                nc.sync.dma_start(out=Sv[0:s, j], in_=Xv[0:s, c])

            out_r = out.rearrange("b k oh ow -> k oh b ow")

            # no-shift kernels first so compute starts before copies finish
            for plan in sorted(plans, key=lambda p: (p[2][0] or p[3][0], p[0])):
                k, base, a_src, b_src, path, scal = plan
                cnt = base + OH  # compute lanes [0, cnt); junk lanes [0, base)

                def view(src):
                    shifted, idx, woff = src
                    tile = Sv if shifted else Xv
                    return tile[0:cnt, idx, :, woff : woff + OW]

                Av, Bv = view(a_src), view(b_src)
                b2 = bp.tile([H, BPC * OW], f32, tag="b2", name=f"b2_{k}")
                b2v = b2.rearrange("p (b w) -> p b w", b=BPC)[0:cnt]
                t = tp.tile([H, BPC * OW], f32, tag="t", name=f"t_{k}")
                tv = t.rearrange("p (b w) -> p b w", b=BPC)[0:cnt]

                if path == "fact":
                    kab, kka, alpha, gamma = scal
                    nc.scalar.activation(b2v, Bv, Copy, bias=kka, scale=kab)
                    nc.vector.scalar_tensor_tensor(tv, Av, alpha, b2v, add, mult)
                    nc.gpsimd.tensor_scalar(tv, tv, gamma, None, add)
                elif path == "linear":
                    kka, kkb, kk1 = scal
                    nc.scalar.activation(b2v, Bv, Copy, bias=kk1, scale=kkb)
                    nc.vector.scalar_tensor_tensor(tv, Av, kka, b2v, mult, add)
                else:  # exact: t = (Ca*A + (Cb*B+C1)) + (Cab*B)*A
                    kab, kka, kkb, kk1 = scal
                    nc.scalar.activation(b2v, Bv, Copy, bias=kk1, scale=kkb)
                    nc.vector.scalar_tensor_tensor(tv, Av, kka, b2v, mult, add)
                    p2 = bp.tile([H, BPC * OW], f32, tag="b2", name=f"p2_{k}")
                    p2v = p2.rearrange("p (b w) -> p b w", b=BPC)[0:cnt]
                    nc.vector.scalar_tensor_tensor(p2v, Bv, kab, Av, mult, mult)
                    nc.gpsimd.tensor_tensor(tv, tv, p2v, add)

                nc.sync.dma_start(out=out_r[k], in_=tv[base : base + OH])
    nc.compile()
    return nc


def kernel(x, pairs_a, pairs_b, weights):
    from concourse.bass_utils import run_bass_kernel_spmd

    x = np.ascontiguousarray(np.asarray(x), dtype=np.float32)
    pa = np.asarray(pairs_a).astype(np.int64)
    pb = np.asarray(pairs_b).astype(np.int64)
    w = np.asarray(weights).astype(np.float32)

    nc = _build(pa, pb, w)
    in_maps = [{"x": x[i * BPC : (i + 1) * BPC]} for i in range(NCORES)]
    res = run_bass_kernel_spmd(nc, in_maps, core_ids=list(range(NCORES)))
    return np.concatenate([r["out"] for r in res.results], axis=0)


# revision 12
# speedup vs baseline: 2.1725x; 2.1725x over previous
"""Trainium2 Bass kernel for nn_LogicConvSparseMatrix.

Math: the reference's 15-term weighted logic-op sum collapses to

    out[b,k] = C_ab[k]*A*B + C_a[k]*A + C_b[k]*B + C_1[k]

where A = x[b, ca_k, ha_k+oh, wa_k+ow], B = x[b, cb_k, hb_k+oh, wb_k+ow]
are shifted 126x126 windows.  With alpha = C_b/C_ab, gamma = C_1 -
C_a*C_b/C_ab this factors into

    out = (A + alpha) * (C_ab*B + C_a) + gamma

Per kernel k (three element passes; two ops cannot carry 4 coefficients):
  1. ScalarE affine:  B2 = C_ab*B + C_a
  2. VectorE scalar_tensor_tensor:  T = (A + alpha) * B2
  3. "+gamma", load-balanced per group of 8 k's across:
       - ScalarE Copy(T*1 + gamma) in place,
       - VectorE tensor_scalar add (AP shaped [4,63] to force 1x mode so
         it never grabs the DVE/GpSimd shared SBUF port), or
       - GpSimd tensor_tensor T + gcol (broadcast gamma table; its
         tensor_scalar kernel is pathologically slow, tensor_tensor is ok).

Index pairs are known at build time, so gathers are compile-time SBUF
views of X[p=h, (c,b,w)].  Compute-engine SBUF operands may only start
at partition 0/32/64/96; the relative h-shift between the two windows is
materialized as per-channel shifted column copies via SBUF->SBUF DMA
(DMA may address any partition).  All compute APs start at partition 0;
store DMAs select rows [base : base+126].

k's are processed sorted by base so stores batch into ~0.5MB group DMAs
(on the Activation HWDGE queue; loads/shifts on the SP queue); the host
inverse-permutes the k axis at the end.
Sharding: data-parallel over batch, 2 batch items per core, 8 cores.
"""

import numpy as np

B, C, H, W = 16, 64, 128, 128
K = 128
RH = RW = 3
OH, OW = H - RH + 1, W - RW + 1
NCORES = 8
BPC = B // NCORES

GRP = 8  # kernels per store group
# gamma-engine per group, round-robin: GpSimd-heavy, DVE/ACT fill
GSPLIT = ("gp", "gp", "dve", "act")


def _coeffs(weights):
    """Per-kernel coefficients of out = Cab*a*b + Ca*a + Cb*b + C1."""
    w = [weights[:, i].astype(np.float64) for i in range(16)]
    cab = w[1] - w[2] - w[4] - 2 * w[6] - w[7] + w[8] + 2 * w[9] + w[11] + w[13] - w[14]
    ca = w[2] + w[3] + w[6] + w[7] - w[8] - w[9] - w[12] - w[13]
    cb = w[4] + w[5] + w[6] + w[7] - w[8] - w[9] - w[10] - w[11]
    c1 = w[8] + w[9] + w[10] + w[11] + w[12] + w[13] + w[14] + w[15]
    return cab, ca, cb, c1


def _plan(pairs_a, pairs_b, weights):
    """Host-side schedule: per-k operand sources/shifts/path, shifted-column
    table (greedy reuse), base-sorted order, broadcast gamma table."""
    cab, ca, cb, c1 = _coeffs(weights)
    cols = {}  # (shift, chan) -> column index in the shifted tile; shift != 0
    plans = []
    for k in range(K):
        ha, wa, cca = int(pairs_a[k][0]), int(pairs_a[k][1]), int(pairs_a[k][2])
        hb, wb, ccb = int(pairs_b[k][0]), int(pairs_b[k][1]), int(pairs_b[k][2])
        # (from_shifted, col_or_chan, w_off) per side; base = h of unshifted side
        if ha == hb:
            base = ha
            a_src, b_src = (False, cca, wa), (False, ccb, wb)
        else:
            # shifting either side keeps that copy's invalid rows inside the
            # junk-lane range (min_h + |delta| <= 2); reuse existing columns.
            if ha < hb:  # a is the smaller-h side
                neg = ((ha - hb, cca), True, hb)  # (col key, shifts_a, base)
                pos = ((hb - ha, ccb), False, ha)
            else:
                neg = ((hb - ha, ccb), False, ha)
                pos = ((ha - hb, cca), True, hb)
            key, shift_a, base = pos if (pos[0] in cols and neg[0] not in cols) else neg
            j = cols.setdefault(key, len(cols))
            if shift_a:
                a_src, b_src = (True, j, wa), (False, ccb, wb)
            else:
                a_src, b_src = (False, cca, wa), (True, j, wb)

        kab, kka, kkb, kk1 = float(cab[k]), float(ca[k]), float(cb[k]), float(c1[k])
        if abs(kab) <= 1e-7:
            path, scal, gamma = "linear", (kka, kkb, kk1), 0.0
        elif abs(kkb) <= 50.0 * abs(kab) and abs(kka * kkb) <= 50.0 * abs(kab):
            path = "fact"
            scal = (kab, kka, kkb / kab)
            gamma = kk1 - kka * kkb / kab
        else:
            path, scal, gamma = "exact", (kab, kka, kkb, kk1), 0.0
        plans.append((k, base, a_src, b_src, path, scal, gamma))

    order = sorted(range(K), key=lambda k: (plans[k][1], k))  # by base
    gcol = np.zeros((H, K), np.float32)
    for pos, k in enumerate(order):
        gcol[:, pos] = plans[k][6]
    return plans, cols, order, gcol


def _build(pairs_a, pairs_b, weights):
    import concourse.bacc as bacc
    import concourse.mybir as mybir
    from concourse.tile import TileContext

    f32 = mybir.dt.float32
    Copy = mybir.ActivationFunctionType.Copy
    add, mult = mybir.AluOpType.add, mybir.AluOpType.mult

    plans, cols, order, gcol_np = _plan(pairs_a, pairs_b, weights)
    ncols = max(1, len(cols))
    ngrp = (K + GRP - 1) // GRP

    if ncols > 85:
        raise RuntimeError(f"shifted-column budget exceeded: {ncols}")

    nc = bacc.Bacc()
    x = nc.dram_tensor("x", [BPC, C, H, W], f32, kind="ExternalInput")
    gcd = nc.dram_tensor("gcol", [H, K], f32, kind="ExternalInput")
    out = nc.dram_tensor("out", [BPC, K, OH, OW], f32, kind="ExternalOutput")

    with TileContext(nc) as tc:
        with (
            tc.tile_pool(name="xp", bufs=1) as xp,
            tc.tile_pool(name="bp", bufs=4) as bp,
            tc.tile_pool(name="tp", bufs=2) as tp,
            tc.tile_pool(name="op", bufs=2) as op,
        ):
            X = xp.tile([H, C * BPC * W], f32)
            Xv = X.rearrange("p (c b w) -> p c b w", c=C, b=BPC)
            for b in range(BPC):
                nc.sync.dma_start(out=Xv[:, :, b], in_=x[b].rearrange("c h w -> h c w"))

            S = xp.tile([H, ncols * BPC * W], f32)
            Sv = S.rearrange("p (j b w) -> p j b w", j=ncols, b=BPC)
            # finite filler for shifted-copy head/tail rows (junk lanes only)
            nc.sync.dma_start(out=Sv[0:2], in_=Xv[0:2, 0:ncols])
            nc.sync.dma_start(out=Sv[H - 2 : H], in_=Xv[0:2, 0:ncols])
            for (s, c), j in cols.items():
                if s < 0:  # S[p] = x[c, p+s]
                    nc.sync.dma_start(out=Sv[-s:H, j], in_=Xv[0 : H + s, c])
                else:
                    nc.sync.dma_start(out=Sv[0 : H - s, j], in_=Xv[s:H, c])

            Gc = xp.tile([H, K], f32)
            nc.sync.dma_start(out=Gc, in_=gcd[:, :])

            out_r = out.rearrange("b k oh ow -> k oh b ow")

            for g in range(ngrp):
                ks = order[g * GRP : (g + 1) * GRP]
                geng = GSPLIT[g % len(GSPLIT)]
                T = tp.tile([H, GRP * BPC * OW], f32, tag="t", name=f"t_{g}")
                if geng == "gp":
                    O = op.tile([H, GRP * BPC * OW], f32, tag="o", name=f"o_{g}")

                for j, k in enumerate(ks):
                    _, base, a_src, b_src, path, scal, gamma = plans[k]
                    cnt = base + OH

                    def view(src):
                        shifted, idx, woff = src
                        t = Sv if shifted else Xv
                        return t[0:cnt, idx, :, woff : woff + OW]

                    Av, Bv = view(a_src), view(b_src)
                    fd = BPC * OW
                    slot = T[0:cnt, j * fd : (j + 1) * fd]
                    slotv = slot.rearrange("p (b w) -> p b w", b=BPC)
                    b2 = bp.tile([H, fd], f32, tag="b2", name=f"b2_{k}")
                    b2v = b2.rearrange("p (b w) -> p b w", b=BPC)[0:cnt]

                    if path == "fact":
                        kab, kka, alpha = scal
                        nc.scalar.activation(b2v, Bv, Copy, bias=kka, scale=kab)
                        nc.vector.scalar_tensor_tensor(slotv, Av, alpha, b2v, add, mult)
                    else:  # linear/exact: slot = Ca*A + (Cb*B + C1)
                        if path == "linear":
                            kka, kkb, kk1 = scal
                        else:
                            kab, kka, kkb, kk1 = scal
                        nc.scalar.activation(b2v, Bv, Copy, bias=kk1, scale=kkb)
                        nc.vector.scalar_tensor_tensor(slotv, Av, kka, b2v, mult, add)
                        if path == "exact":  # += (Cab*B)*A
                            p2 = bp.tile([H, fd], f32, tag="b2", name=f"p2_{k}")
                            p2v = p2.rearrange("p (b w) -> p b w", b=BPC)[0:cnt]
                            nc.vector.scalar_tensor_tensor(p2v, Bv, kab, Av, mult, mult)
                            nc.vector.tensor_tensor(slot, slot, p2[0:cnt], add)

                    # the +gamma pass (skipped where gamma == 0)
                    if gamma != 0.0 or geng == "gp":
                        pos = g * GRP + j
                        if geng == "act":
                            nc.scalar.activation(slot, slot, Copy, bias=gamma, scale=1.0)
                        elif geng == "dve":
                            # odd innermost dim forces 1x mode: no shared-port
                            # contention with GpSimd
                            so = slot.rearrange("p (a q) -> p a q", a=4)
                            nc.vector.tensor_scalar(so, so, gamma, None, add)
                        else:
                            gb = Gc[0:cnt, pos : pos + 1].broadcast_to([cnt, fd])
                            osl = O[0:cnt, j * fd : (j + 1) * fd]
                            nc.gpsimd.tensor_tensor(osl, slot, gb, add)

                # batched stores per same-base run (Activation HWDGE queue)
                src_t = O if geng == "gp" else T
                i = 0
                while i < len(ks):
                    base = plans[ks[i]][1]
                    i2 = i
                    while i2 < len(ks) and plans[ks[i2]][1] == base:
                        i2 += 1
                    fd = BPC * OW
                    for b in range(BPC):
                        src = src_t.rearrange("p (j q) -> p j q", j=GRP)[
                            base : base + OH, i:i2, b * OW : (b + 1) * OW
                        ]
                        dst = out_r[g * GRP + i : g * GRP + i2, :, b]
                        nc.scalar.dma_start(
                            out=dst.rearrange("k oh ow -> oh k ow"), in_=src
                        )
                    i = i2
    nc.compile()
    return nc


def _consts(pairs_a, pairs_b, weights):
    plans, cols, order, gcol = _plan(pairs_a, pairs_b, weights)
    return {"gcol": gcol}, order


def kernel(x, pairs_a, pairs_b, weights):
    from concourse.bass_utils import run_bass_kernel_spmd

    x = np.ascontiguousarray(np.asarray(x), dtype=np.float32)
    pa = np.asarray(pairs_a).astype(np.int64)
    pb = np.asarray(pairs_b).astype(np.int64)
    w = np.asarray(weights).astype(np.float32)

    nc = _build(pa, pb, w)
    extra, order = _consts(pa, pb, w)
    in_maps = [{"x": x[i * BPC : (i + 1) * BPC], **extra} for i in range(NCORES)]
    res = run_bass_kernel_spmd(nc, in_maps, core_ids=list(range(NCORES)))
    out_dev = np.concatenate([r["out"] for r in res.results], axis=0)
    pos = np.empty(K, np.int64)
    pos[np.asarray(order)] = np.arange(K)
    return out_dev[:, pos]
